# revision 29
# baseline (speedup 1.0000x reference)
"""Butterfly (Givens) rotation network on TRN2, 8 NeuronCores.

Algorithm
---------
x: (8192, 4096) f32. 12 butterfly layers; layer l rotates pairs of features
differing in bit l of the feature index. Split into two linear stages:

  Stage A = layers 0-6: features mix only within 128-wide blocks b (bits 0-6)
            -> per-block 128x128 matrix A_b  (amats[:, 128b:128b+128] =
            A_b[f_in, f_out]).
  Stage B = layers 7-11: features mix only across blocks at fixed within-block
            position p (bits 7-11) -> per-p 32x32 matrix B_p; grouping 4
            consecutive p per 128-partition tile gives block-diag 128x128
            (bmats tile t, within-tile index n = pl*32 + b for p = 4t+pl).

Variants
--------
v1: original fp32 row-major kernel (PE transposes + data-stationary matmuls).
v3: v1 structure, all-bf16 (PE transpose 1cyc/row, matmul 1cyc/row vs 4).
v4: feature-major, DMA-xbar-transposed load (zero stage-A PE transposes),
    weights-stationary bf16 matmuls at N=256, partition-regroup between the
    stages done by a plain SBUF->SBUF DMA, output left feature-major and
    unscrambled on the host.

Sharding: data-parallel over rows, 1024 rows/core; matrices replicated.
"""

import os
import numpy as np
import ml_dtypes

BF16 = ml_dtypes.bfloat16

DIM = 4096
NL = 12
NB = 32          # 128-wide feature blocks
ROWS = 8192
NCORES = 8
RPC = ROWS // NCORES     # rows per core
NT = RPC // 128          # 128-row tiles per core

RPS = 512                # v4: rows per slab
NSLAB = RPC // RPS       # v4: slabs per core


# ---------------------------------------------------------------- host math

def _butterfly_np(x, angles):
    """float64 numpy copy of the reference butterfly."""
    x = np.asarray(x, np.float64)
    angles = np.asarray(angles, np.float64)
    B, d = x.shape
    for l in range(angles.shape[0]):
        stride = 2 ** l
        nblocks = d // (2 * stride)
        xr = x.reshape(B, nblocks, 2, stride)
        c = np.cos(angles[l]).reshape(nblocks, stride)
        s = np.sin(angles[l]).reshape(nblocks, stride)
        xi = xr[:, :, 0, :].copy()
        xj = xr[:, :, 1, :].copy()
        x = np.stack([c * xi + s * xj, -s * xi + c * xj], axis=2).reshape(B, d)
    return x


def _build_mats(angles):
    """Returns (amats, bmats) each [128, 4096] f64 in SBUF-ready layout."""
    angles = np.asarray(angles, np.float64)
    amats = np.zeros((128, DIM), np.float64)
    for b in range(NB):
        # A_b[f_in, f_out]: butterfly of identity rows = F for this block
        amats[:, 128 * b:128 * b + 128] = _butterfly_np(
            np.eye(128), angles[0:7, 64 * b:64 * b + 64])
    bmats = np.zeros((128, DIM), np.float64)
    for t in range(32):
        for pl in range(4):
            p = 4 * t + pl
            BpT = _butterfly_np(np.eye(32), angles[7:12, p::128])
            bmats[32 * pl:32 * pl + 32, 128 * t + 32 * pl:128 * t + 32 * pl + 32] = BpT
    return amats, bmats


# ---------------------------------------------------------------- bass kernels

def _emit_kernel_v3(ctx, tc, out, x, amats, bmats, ident):
    """v1 structure, all-bf16: per 128-row tile, PE-transpose each feature
    block, bf16 matmul against A (data stationary), scatter-drain into f~
    order, repeat for stage B, DMA out bf16."""
    import concourse.mybir as mybir

    nc = tc.nc
    f32 = mybir.dt.float32
    bf16 = mybir.dt.bfloat16

    consts = ctx.enter_context(tc.tile_pool(name="consts", bufs=1))
    xin = ctx.enter_context(tc.tile_pool(name="xin", bufs=3))
    ystage = ctx.enter_context(tc.tile_pool(name="ystage", bufs=3))
    ostage = ctx.enter_context(tc.tile_pool(name="ostage", bufs=3))
    sbst = ctx.enter_context(tc.tile_pool(name="sbst", bufs=6))
    psA = ctx.enter_context(tc.tile_pool(name="psA", bufs=4, space="PSUM"))
    psB = ctx.enter_context(tc.tile_pool(name="psB", bufs=4, space="PSUM"))

    am = consts.tile([128, DIM], bf16, tag="amats")
    bm = consts.tile([128, DIM], bf16, tag="bmats")
    idt = consts.tile([128, 128], bf16, tag="ident")
    nc.sync.dma_start(idt[:], ident[:])

    # Greedy least-loaded assignment of PSUM->SBUF copies to DVE/ACT,
    # using measured per-copy costs (ns) for [128,512] from PSUM.
    load = {"dve": 0.0, "act": 0.0}
    cost = {("dve", "plain"): 392, ("dve", "scatter"): 700,
            ("act", "plain"): 357, ("act", "scatter"): 1127}

    def copy(dst, src, kind="plain"):
        eng = min(("dve", "act"), key=lambda e: load[e] + cost[(e, kind)])
        load[eng] += cost[(eng, kind)]
        (nc.vector.tensor_copy if eng == "dve" else nc.scalar.copy)(dst, src)

    for i in range(NT):
        xt = xin.tile([128, DIM], bf16, tag="xt")
        if i == 0:
            # first tile: fine-grained x/amats chunk interleave so the very
            # first transposes and stage-A matmuls start as early as possible
            for c in range(8):
                nc.sync.dma_start(xt[:, 512 * c:512 * (c + 1)],
                                  x[0:128, 512 * c:512 * (c + 1)])
                nc.sync.dma_start(am[:, 512 * c:512 * (c + 1)],
                                  amats[:, 512 * c:512 * (c + 1)])
        else:
            nc.sync.dma_start(xt[:], x[128 * i:128 * (i + 1), :])
        Y = ystage.tile([128, DIM], bf16, tag="Y")

        for g in range(8):           # groups of 4 feature blocks
            pt = psA.tile([128, 512], bf16, tag="ptA")
            for j in range(4):
                b = 4 * g + j
                nc.tensor.transpose(
                    pt[:, 128 * j:128 * (j + 1)],
                    xt[:, 128 * b:128 * (b + 1)], idt[:])
            xT4 = sbst.tile([128, 512], bf16, tag="xT4")
            copy(xT4[:], pt[:])
            pm = psB.tile([128, 512], f32, tag="pmA")
            for j in range(4):
                b = 4 * g + j
                nc.tensor.matmul(
                    pm[:, 128 * j:128 * (j + 1)],
                    xT4[:, 128 * j:128 * (j + 1)],
                    am[:, 128 * b:128 * (b + 1)],
                    start=True, stop=True)
            # scatter into Y: dest f~ = t*128 + pl*32 + (4g+j), src = j*128 + 4t + pl
            src = pm[:].rearrange("r (j t pl) -> r j t pl", j=4, t=32, pl=4)
            dst = Y[:].rearrange(
                "r (t pl g j) -> r g j t pl", t=32, pl=4, g=8, j=4)[:, g]
            copy(dst, src, kind="scatter")

        if i == 0:
            for c in range(8):
                nc.sync.dma_start(bm[:, 512 * c:512 * (c + 1)],
                                  bmats[:, 512 * c:512 * (c + 1)])
        O = ostage.tile([128, DIM], bf16, tag="O")
        for g in range(8):           # groups of 4 f~ tiles
            pt = psA.tile([128, 512], bf16, tag="ptA")
            for j in range(4):
                t = 4 * g + j
                nc.tensor.transpose(
                    pt[:, 128 * j:128 * (j + 1)],
                    Y[:, 128 * t:128 * (t + 1)], idt[:])
            z4 = sbst.tile([128, 512], bf16, tag="xT4")
            copy(z4[:], pt[:])
            pm = psB.tile([128, 512], f32, tag="pmA")
            for j in range(4):
                t = 4 * g + j
                nc.tensor.matmul(
                    pm[:, 128 * j:128 * (j + 1)],
                    z4[:, 128 * j:128 * (j + 1)],
                    bm[:, 128 * t:128 * (t + 1)],
                    start=True, stop=True)
            # scatter to natural order: dest f = b*128 + 4t + pl = b*128 + 16g + 4j + pl
            src = pm[:].rearrange("r (j pl b) -> r j pl b", j=4, pl=4, b=32)
            dst = O[:].rearrange(
                "r (b g j pl) -> r g j pl b", b=32, g=8, j=4, pl=4)[:, g]
            copy(dst, src, kind="scatter")

        nc.sync.dma_start(out[128 * i:128 * (i + 1), :], O[:])


def _emit_kernel_v4(ctx, tc, out, x, amats, bmats):
    """Feature-major pipeline, mixed-granularity regroup:
      quartered loads (SP) -> stage A both slabs into ONE Y [128, (b,s,r)]
      (one-bank drains, psA bufs=4) -> regroup: 18 spread tiles as FULL-CORE
      DMAs on SP (HWDGE cost is descriptor-count-bound, so 2x payload is
      free), remaining 14 tiles per slab on the 4 GPSIMD/SWDGE rings ->
      stage B ordered to match completion (GP slab 0, SP both-slab tiles,
      GP slab 1) -> W per slab (reuses dead XT buffers) -> stores (SP).
      Host unscrambles."""
    import concourse.mybir as mybir

    nc = tc.nc
    f32 = mybir.dt.float32
    bf16 = mybir.dt.bfloat16

    consts = ctx.enter_context(tc.tile_pool(name="consts", bufs=1))
    xwp = ctx.enter_context(tc.tile_pool(name="xwp", bufs=2))
    ypool = ctx.enter_context(tc.tile_pool(name="ypool", bufs=1))
    zsp = ctx.enter_context(tc.tile_pool(name="zsp", bufs=6))
    zgp = ctx.enter_context(tc.tile_pool(name="zgp", bufs=20))
    psA = ctx.enter_context(tc.tile_pool(name="psA", bufs=4, space="PSUM"))
    psB = ctx.enter_context(tc.tile_pool(name="psB", bufs=4, space="PSUM"))

    am = consts.tile([128, DIM], bf16, tag="amats")
    bm = consts.tile([128, DIM], bf16, tag="bmats")
    nc.sync.dma_start(am[:], amats[:])
    nc.sync.dma_start(bm[:], bmats[:])

    QC = NB // 4                   # feature blocks per quarter-DMA
    XTs = []
    for s in range(NSLAB):
        XT = xwp.tile([128, NB * RPS], bf16, tag="XW", name=f"XT{s}")
        for h in range(4):
            nc.sync.dma_start(XT[:, QC * RPS * h:QC * RPS * (h + 1)],
                              x[s, :, QC * h:QC * (h + 1)])
        XTs.append(XT)

    load = {"dve": 0.0, "act": 0.0}
    cost = {"dve": 1.2, "act": 1.0}

    def copy(dst, src):
        eng = min(("dve", "act"), key=lambda e: load[e] + cost[e])
        load[eng] += cost[eng]
        (nc.vector.tensor_copy if eng == "dve" else nc.scalar.copy)(dst, src)

    BC = NSLAB * RPS               # Y columns per feature block
    Y = ypool.tile([128, NB * BC], bf16, tag="Y")
    for s in range(NSLAB):
        XT = XTs[s]
        for b in range(NB):
            pm = psA.tile([128, RPS], f32, tag="pmA")
            nc.tensor.matmul(
                pm[:], am[:, 128 * b:128 * (b + 1)],
                XT[:, RPS * b:RPS * (b + 1)], start=True, stop=True)
            copy(Y[:, BC * b + RPS * s:BC * b + RPS * (s + 1)], pm[:])

    # 18 full-core tiles on SP (3us gen each), 14/slab on GPSIMD (2us each)
    SPT = sorted({round(i * 32 / 18) for i in range(18)})
    GPT = [t for t in range(NB) if t not in set(SPT)]
    Ws = [xwp.tile([128, NB * RPS], bf16, tag="XW", name=f"W{s}")
          for s in range(NSLAB)]

    def bmm(t, s, rhs):
        pm = psB.tile([128, RPS], f32, tag="pmB")
        nc.tensor.matmul(
            pm[:], bm[:, 128 * t:128 * (t + 1)], rhs, start=True, stop=True)
        copy(Ws[s][:, RPS * t:RPS * (t + 1)], pm[:])

    # GP slab-0 regroups + their B matmuls
    gz0 = {}
    for t in GPT:
        Zt = zgp.tile([128, RPS], bf16, tag="Zg")
        srcap = Y[4 * t:4 * t + 4, :].rearrange(
            "p (b s r) -> p b s r", b=NB, s=NSLAB, r=RPS)[:, :, 0]
        nc.gpsimd.dma_start(Zt[:], srcap)
        gz0[t] = Zt
    # SP full-core regroups
    spz = {}
    for t in SPT:
        Zt = zsp.tile([128, BC], bf16, tag="Zs")
        nc.sync.dma_start(Zt[:], Y[4 * t:4 * t + 4, :])
        spz[t] = Zt
    # GP slab-1 regroups
    gz1 = {}
    for t in GPT:
        Zt = zgp.tile([128, RPS], bf16, tag="Zg")
        srcap = Y[4 * t:4 * t + 4, :].rearrange(
            "p (b s r) -> p b s r", b=NB, s=NSLAB, r=RPS)[:, :, 1]
        nc.gpsimd.dma_start(Zt[:], srcap)
        gz1[t] = Zt

    for t in GPT:
        bmm(t, 0, gz0[t][:])
    for t in SPT:
        for s in range(NSLAB):
            bmm(t, s, spz[t][:, RPS * s:RPS * (s + 1)])
    for t in GPT:
        bmm(t, 1, gz1[t][:])

    for s in range(NSLAB):
        for h in range(4):
            nc.sync.dma_start(out[s, :, QC * h:QC * (h + 1)],
                              Ws[s][:, QC * RPS * h:QC * RPS * (h + 1)])


def _hoist_matmul_waits(nc):
    """Walrus's fp32/transpose matmul (self-loading LDWEIGHTS) accepts fewer
    sync waits than Tile may assign. Hoist multi-waits onto a PE NoOp inserted
    just before the matmul — same engine queue, so ordering is identical."""
    import concourse.mybir as mybir

    n_hoisted = 0
    for blk in nc.m.functions[0].blocks:
        il = blk.instructions
        i = 0
        while i < len(il):
            inst = il[i]
            si = inst.sync_info
            if (si is not None and len(si.on_wait) > 1
                    and not isinstance(inst, mybir.InstNoOp)):
                waits = list(si.on_wait)
                # keep the last wait on the matmul; one NoOp per extra wait
                # (cayman instructions carry at most one sem-wait each)
                for k, w in enumerate(waits[:-1]):
                    nop = mybir.InstNoOp(
                        name=f"{inst.name}_hw{k}", engine=inst.engine,
                        bass_nofuse=True)
                    nop.sync_info = mybir.SyncInfo(on_wait=[w], on_update=[])
                    nc.register_instruction(nop, overwrite=True)
                    il.insert(i, nop)
                    i += 1
                    n_hoisted += 1
                inst.sync_info = mybir.SyncInfo(
                    on_wait=[waits[-1]], on_update=list(si.on_update))
            i += 1
    return n_hoisted


def _spread_swdge_queues(nc):
    """Round-robin Pool-engine DMA copies across the SWDGE rings so the SDMA
    engines interleave two regroups (packet-granular round-robin between
    queues) and their partition-concentrated reads spread across ports."""
    import concourse.mybir as mybir

    n = 0
    for blk in nc.m.functions[0].blocks:
        for inst in blk.instructions:
            if (isinstance(inst, mybir.InstDMACopy)
                    and inst.queue == "qPoolDynamic"):
                if n % 4:
                    inst.queue = f"qPoolDynamic{n % 4}"
                n += 1
    return n


_CACHED = {}
VARIANT = os.environ.get("BFLY_VARIANT", "v4")   # "v3" | "v4"


def _build_bass(variant=None):
    variant = variant or VARIANT
    if variant in _CACHED:
        return _CACHED[variant]
    from contextlib import ExitStack
    import concourse.bass as bass
    import concourse.tile as tile
    import concourse.mybir as mybir

    bf16 = mybir.dt.bfloat16
    nc = bass.Bass("TRN2", target_bir_lowering=False, debug=False,
                   num_devices=NCORES, num_swdge_queues=4)
    xshape = [RPC, DIM] if variant == "v3" else [NSLAB, 128, NB, RPS]
    x = nc.dram_tensor("x", xshape, bf16, kind="ExternalInput").ap()
    amats = nc.dram_tensor("amats", [128, DIM], bf16, kind="ExternalInput").ap()
    bmats = nc.dram_tensor("bmats", [128, DIM], bf16, kind="ExternalInput").ap()

    with tile.TileContext(nc) as tc:
        with ExitStack() as ctx:
            if variant == "v3":
                ident = nc.dram_tensor(
                    "ident", [128, 128], bf16, kind="ExternalInput").ap()
                out = nc.dram_tensor(
                    "out", [RPC, DIM], bf16, kind="ExternalOutput").ap()
                _emit_kernel_v3(ctx, tc, out, x, amats, bmats, ident)
            else:
                out = nc.dram_tensor(
                    "out", [NSLAB, 128, NB, RPS], bf16,
                    kind="ExternalOutput").ap()
                _emit_kernel_v4(ctx, tc, out, x, amats, bmats)

    _hoist_matmul_waits(nc)
    _spread_swdge_queues(nc)
    _CACHED[variant] = nc
    return nc


def make_in_maps(x, angles, variant=None):
    variant = variant or VARIANT
    x = np.ascontiguousarray(np.asarray(x, np.float32)).astype(BF16)
    amats64, bmats64 = _build_mats(angles)
    amats = amats64.astype(BF16)
    bmats = bmats64.astype(BF16)
    maps = []
    for c in range(NCORES):
        xc = x[c * RPC:(c + 1) * RPC]
        if variant != "v3":
            # pretranspose/pretile: [s, p, b, r] = feature b*128+p, row s*RPS+r
            xc = np.ascontiguousarray(
                xc.reshape(NSLAB, RPS, NB, 128).transpose(0, 3, 2, 1))
        m = {"x": xc, "amats": amats, "bmats": bmats}
        if variant == "v3":
            m["ident"] = np.eye(128, dtype=BF16)
        maps.append(m)
    return maps


def _unscramble(res, variant=None):
    variant = variant or VARIANT
    outs = []
    for c in range(NCORES):
        o = np.asarray(res.results[c]["out"]).astype(np.float32)
        if variant == "v3":
            outs.append(o)
        else:
            # o[s, pl*32+b, t*RPS+r] -> row s*RPS+r, feature b*128+4t+pl
            o = o.reshape(NSLAB, 4, 32, 32, RPS)
            outs.append(np.transpose(o, (0, 4, 2, 3, 1)).reshape(RPC, DIM))
    return np.concatenate(outs, axis=0)


def run_on_hw(x, angles, trace=False, trace_kwargs=None):
    from concourse.bass_utils import run_bass_kernel_spmd
    nc = _build_bass()
    in_maps = make_in_maps(x, angles)
    res = run_bass_kernel_spmd(
        nc, in_maps, core_ids=list(range(NCORES)), trace=trace,
        **(trace_kwargs or {}))
    out = _unscramble(res)
    return out, res


def kernel(x, angles):
    last_err = None
    for attempt in range(3):
        try:
            out, _ = run_on_hw(x, angles, trace=False)
            return np.ascontiguousarray(out.astype(np.float32))
        except Exception as e:  # transient NRT/device errors: retry
            last_err = e
            import time
            time.sleep(5)
    raise last_err


# revision 31
# speedup vs baseline: 1.2654x; 1.2654x over previous
"""Butterfly (Givens) rotation network on TRN2, 8 NeuronCores.

Algorithm
---------
x: (8192, 4096) f32. 12 butterfly layers; layer l rotates pairs of features
differing in bit l of the feature index. Split into two linear stages:

  Stage A = layers 0-6: features mix only within 128-wide blocks b (bits 0-6)
            -> per-block 128x128 matrix A_b  (amats[:, 128b:128b+128] =
            A_b[f_in, f_out]).
  Stage B = layers 7-11: features mix only across blocks at fixed within-block
            position p (bits 7-11) -> per-p 32x32 matrix B_p; grouping 4
            consecutive p per 128-partition tile gives block-diag 128x128
            (bmats tile t, within-tile index n = pl*32 + b for p = 4t+pl).

Variants
--------
v1: original fp32 row-major kernel (PE transposes + data-stationary matmuls).
v3: v1 structure, all-bf16 (PE transpose 1cyc/row, matmul 1cyc/row vs 4).
v4: feature-major, DMA-xbar-transposed load (zero stage-A PE transposes),
    weights-stationary bf16 matmuls at N=256, partition-regroup between the
    stages done by a plain SBUF->SBUF DMA, output left feature-major and
    unscrambled on the host.

Sharding: data-parallel over rows, 1024 rows/core; matrices replicated.
"""

import os
import numpy as np
import ml_dtypes

BF16 = ml_dtypes.bfloat16

DIM = 4096
NL = 12
NB = 32          # 128-wide feature blocks
ROWS = 8192
NCORES = 8
RPC = ROWS // NCORES     # rows per core
NT = RPC // 128          # 128-row tiles per core

RPS = 512                # v4: rows per slab
NSLAB = RPC // RPS       # v4: slabs per core


# ---------------------------------------------------------------- host math

def _butterfly_np(x, angles):
    """float64 numpy copy of the reference butterfly."""
    x = np.asarray(x, np.float64)
    angles = np.asarray(angles, np.float64)
    B, d = x.shape
    for l in range(angles.shape[0]):
        stride = 2 ** l
        nblocks = d // (2 * stride)
        xr = x.reshape(B, nblocks, 2, stride)
        c = np.cos(angles[l]).reshape(nblocks, stride)
        s = np.sin(angles[l]).reshape(nblocks, stride)
        xi = xr[:, :, 0, :].copy()
        xj = xr[:, :, 1, :].copy()
        x = np.stack([c * xi + s * xj, -s * xi + c * xj], axis=2).reshape(B, d)
    return x


def _build_mats(angles):
    """Returns (amats, bmats) each [128, 4096] f64 in SBUF-ready layout."""
    angles = np.asarray(angles, np.float64)
    amats = np.zeros((128, DIM), np.float64)
    for b in range(NB):
        # A_b[f_in, f_out]: butterfly of identity rows = F for this block
        amats[:, 128 * b:128 * b + 128] = _butterfly_np(
            np.eye(128), angles[0:7, 64 * b:64 * b + 64])
    bmats = np.zeros((128, DIM), np.float64)
    for t in range(32):
        for pl in range(4):
            p = 4 * t + pl
            BpT = _butterfly_np(np.eye(32), angles[7:12, p::128])
            bmats[32 * pl:32 * pl + 32, 128 * t + 32 * pl:128 * t + 32 * pl + 32] = BpT
    return amats, bmats


# ---------------------------------------------------------------- bass kernels

def _emit_kernel_v3(ctx, tc, out, x, amats, bmats, ident):
    """v1 structure, all-bf16: per 128-row tile, PE-transpose each feature
    block, bf16 matmul against A (data stationary), scatter-drain into f~
    order, repeat for stage B, DMA out bf16."""
    import concourse.mybir as mybir

    nc = tc.nc
    f32 = mybir.dt.float32
    bf16 = mybir.dt.bfloat16

    consts = ctx.enter_context(tc.tile_pool(name="consts", bufs=1))
    xin = ctx.enter_context(tc.tile_pool(name="xin", bufs=3))
    ystage = ctx.enter_context(tc.tile_pool(name="ystage", bufs=3))
    ostage = ctx.enter_context(tc.tile_pool(name="ostage", bufs=3))
    sbst = ctx.enter_context(tc.tile_pool(name="sbst", bufs=6))
    psA = ctx.enter_context(tc.tile_pool(name="psA", bufs=5, space="PSUM"))
    psB = ctx.enter_context(tc.tile_pool(name="psB", bufs=3, space="PSUM"))

    am = consts.tile([128, DIM], bf16, tag="amats")
    bm = consts.tile([128, DIM], bf16, tag="bmats")
    idt = consts.tile([128, 128], bf16, tag="ident")
    nc.sync.dma_start(idt[:], ident[:])

    # Greedy least-loaded assignment of PSUM->SBUF copies to DVE/ACT,
    # using measured per-copy costs (ns) for [128,512] from PSUM.
    load = {"dve": 0.0, "act": 0.0}
    cost = {("dve", "plain"): 392, ("dve", "scatter"): 700,
            ("act", "plain"): 357, ("act", "scatter"): 1127}

    def copy(dst, src, kind="plain"):
        eng = min(("dve", "act"), key=lambda e: load[e] + cost[(e, kind)])
        load[eng] += cost[(eng, kind)]
        (nc.vector.tensor_copy if eng == "dve" else nc.scalar.copy)(dst, src)

    for i in range(NT):
        xt = xin.tile([128, DIM], bf16, tag="xt")
        if i == 0:
            # first tile: fine-grained x/amats chunk interleave so the very
            # first transposes and stage-A matmuls start as early as possible
            for c in range(8):
                nc.sync.dma_start(xt[:, 512 * c:512 * (c + 1)],
                                  x[0:128, 512 * c:512 * (c + 1)])
                nc.sync.dma_start(am[:, 512 * c:512 * (c + 1)],
                                  amats[:, 512 * c:512 * (c + 1)])
        else:
            nc.sync.dma_start(xt[:], x[128 * i:128 * (i + 1), :])
        Y = ystage.tile([128, DIM], bf16, tag="Y")

        for g in range(8):           # groups of 4 feature blocks
            pt = psA.tile([128, 512], bf16, tag="ptA")
            for j in range(4):
                b = 4 * g + j
                nc.tensor.transpose(
                    pt[:, 128 * j:128 * (j + 1)],
                    xt[:, 128 * b:128 * (b + 1)], idt[:])
            xT4 = sbst.tile([128, 512], bf16, tag="xT4")
            copy(xT4[:], pt[:])
            pm = psB.tile([128, 512], f32, tag="pmA")
            for j in range(4):
                b = 4 * g + j
                nc.tensor.matmul(
                    pm[:, 128 * j:128 * (j + 1)],
                    xT4[:, 128 * j:128 * (j + 1)],
                    am[:, 128 * b:128 * (b + 1)],
                    start=True, stop=True)
            # scatter into Y: dest f~ = t*128 + pl*32 + (4g+j), src = j*128 + 4t + pl
            src = pm[:].rearrange("r (j t pl) -> r j t pl", j=4, t=32, pl=4)
            dst = Y[:].rearrange(
                "r (t pl g j) -> r g j t pl", t=32, pl=4, g=8, j=4)[:, g]
            copy(dst, src, kind="scatter")

        if i == 0:
            for c in range(8):
                nc.sync.dma_start(bm[:, 512 * c:512 * (c + 1)],
                                  bmats[:, 512 * c:512 * (c + 1)])
        O = ostage.tile([128, DIM], bf16, tag="O")
        for g in range(8):           # groups of 4 f~ tiles
            pt = psA.tile([128, 512], bf16, tag="ptA")
            for j in range(4):
                t = 4 * g + j
                nc.tensor.transpose(
                    pt[:, 128 * j:128 * (j + 1)],
                    Y[:, 128 * t:128 * (t + 1)], idt[:])
            z4 = sbst.tile([128, 512], bf16, tag="xT4")
            copy(z4[:], pt[:])
            pm = psB.tile([128, 512], f32, tag="pmA")
            for j in range(4):
                t = 4 * g + j
                nc.tensor.matmul(
                    pm[:, 128 * j:128 * (j + 1)],
                    z4[:, 128 * j:128 * (j + 1)],
                    bm[:, 128 * t:128 * (t + 1)],
                    start=True, stop=True)
            # scatter to natural order: dest f = b*128 + 4t + pl = b*128 + 16g + 4j + pl
            src = pm[:].rearrange("r (j pl b) -> r j pl b", j=4, pl=4, b=32)
            dst = O[:].rearrange(
                "r (b g j pl) -> r g j pl b", b=32, g=8, j=4, pl=4)[:, g]
            copy(dst, src, kind="scatter")

        nc.sync.dma_start(out[128 * i:128 * (i + 1), :], O[:])


def _emit_kernel_v4(ctx, tc, out, x, amats, bmats):
    """Feature-major pipeline. Emission order matches readiness: quartered
    loads for all slabs (SP), stage A for all slabs (1 matmul + 1 one-bank
    drain per feature block, psA bufs=4), then per slab all 32 regroup DMAs
    (SP/GPSIMD split, 16-deep Z rotation) followed by stage B, stores
    (quartered, SP) last. W reuses the dead XT buffer. Host unscrambles."""
    import concourse.mybir as mybir

    nc = tc.nc
    f32 = mybir.dt.float32
    bf16 = mybir.dt.bfloat16

    consts = ctx.enter_context(tc.tile_pool(name="consts", bufs=1))
    xwp = ctx.enter_context(tc.tile_pool(name="xwp", bufs=2))
    ypool = ctx.enter_context(tc.tile_pool(name="ypool", bufs=2))
    zpool = ctx.enter_context(tc.tile_pool(name="zpool", bufs=28))
    psA = ctx.enter_context(tc.tile_pool(name="psA", bufs=5, space="PSUM"))
    psB = ctx.enter_context(tc.tile_pool(name="psB", bufs=3, space="PSUM"))

    am = consts.tile([128, DIM], bf16, tag="amats")
    bm = consts.tile([128, DIM], bf16, tag="bmats")
    nc.sync.dma_start(am[:], amats[:])
    nc.sync.dma_start(bm[:], bmats[:])

    QC = NB // 4                   # feature blocks per quarter (stores)
    EC = NB // 8                   # feature blocks per eighth (loads)
    XTs = []
    for s in range(NSLAB):
        XT = xwp.tile([128, NB * RPS], bf16, tag="XW", name=f"XT{s}")
        for h in range(8):
            nc.sync.dma_start(XT[:, EC * RPS * h:EC * RPS * (h + 1)],
                              x[s, :, EC * h:EC * (h + 1)])
        XTs.append(XT)

    load = {"dve": 0.0, "act": 0.0}
    cost = {"dve": 1.2, "act": 1.0}

    def copy(dst, src):
        eng = min(("dve", "act"), key=lambda e: load[e] + cost[e])
        load[eng] += cost[eng]
        (nc.vector.tensor_copy if eng == "dve" else nc.scalar.copy)(dst, src)

    Ys = []
    for s in range(NSLAB):
        XT = XTs[s]
        Y = ypool.tile([128, NB * RPS], bf16, tag="Y", name=f"Y{s}")
        for b in range(NB):
            pm = psA.tile([128, RPS], f32, tag="pmA")
            nc.tensor.matmul(
                pm[:], am[:, 128 * b:128 * (b + 1)],
                XT[:, RPS * b:RPS * (b + 1)], start=True, stop=True)
            copy(Y[:, RPS * b:RPS * (b + 1)], pm[:])
        Ys.append(Y)

    Ws = []
    for s in range(NSLAB):
        Y = Ys[s]
        zs = []
        for t in range(NB):
            Zt = zpool.tile([128, RPS], bf16, tag="Zt")
            eng = nc.sync if t % 4 == 0 else nc.gpsimd
            eng.dma_start(Zt[:], Y[4 * t:4 * t + 4, :])
            zs.append(Zt)
        W = xwp.tile([128, NB * RPS], bf16, tag="XW", name=f"W{s}")
        for t in range(NB):
            pm = psB.tile([128, RPS], f32, tag="pmB")
            nc.tensor.matmul(
                pm[:], bm[:, 128 * t:128 * (t + 1)], zs[t][:],
                start=True, stop=True)
            copy(W[:, RPS * t:RPS * (t + 1)], pm[:])
        Ws.append(W)

    for s in range(NSLAB):
        for h in range(4):
            nc.sync.dma_start(out[s, :, QC * h:QC * (h + 1)],
                              Ws[s][:, QC * RPS * h:QC * RPS * (h + 1)])


def _hoist_matmul_waits(nc):
    """Walrus's fp32/transpose matmul (self-loading LDWEIGHTS) accepts fewer
    sync waits than Tile may assign. Hoist multi-waits onto a PE NoOp inserted
    just before the matmul — same engine queue, so ordering is identical."""
    import concourse.mybir as mybir

    n_hoisted = 0
    for blk in nc.m.functions[0].blocks:
        il = blk.instructions
        i = 0
        while i < len(il):
            inst = il[i]
            si = inst.sync_info
            if (si is not None and len(si.on_wait) > 1
                    and not isinstance(inst, mybir.InstNoOp)):
                waits = list(si.on_wait)
                # keep the last wait on the matmul; one NoOp per extra wait
                # (cayman instructions carry at most one sem-wait each)
                for k, w in enumerate(waits[:-1]):
                    nop = mybir.InstNoOp(
                        name=f"{inst.name}_hw{k}", engine=inst.engine,
                        bass_nofuse=True)
                    nop.sync_info = mybir.SyncInfo(on_wait=[w], on_update=[])
                    nc.register_instruction(nop, overwrite=True)
                    il.insert(i, nop)
                    i += 1
                    n_hoisted += 1
                inst.sync_info = mybir.SyncInfo(
                    on_wait=[waits[-1]], on_update=list(si.on_update))
            i += 1
    return n_hoisted


def _spread_swdge_queues(nc):
    """Round-robin Pool-engine DMA copies across the SWDGE rings so the SDMA
    engines interleave two regroups (packet-granular round-robin between
    queues) and their partition-concentrated reads spread across ports."""
    import concourse.mybir as mybir

    n = 0
    for blk in nc.m.functions[0].blocks:
        for inst in blk.instructions:
            if (isinstance(inst, mybir.InstDMACopy)
                    and inst.queue == "qPoolDynamic"):
                if n % 4:
                    inst.queue = f"qPoolDynamic{n % 4}"
                n += 1
    return n


_CACHED = {}
VARIANT = os.environ.get("BFLY_VARIANT", "v4")   # "v3" | "v4"


def _build_bass(variant=None):
    variant = variant or VARIANT
    if variant in _CACHED:
        return _CACHED[variant]
    from contextlib import ExitStack
    import concourse.bass as bass
    import concourse.tile as tile
    import concourse.mybir as mybir

    bf16 = mybir.dt.bfloat16
    nc = bass.Bass("TRN2", target_bir_lowering=False, debug=False,
                   num_devices=NCORES, num_swdge_queues=4)
    xshape = [RPC, DIM] if variant == "v3" else [NSLAB, 128, NB, RPS]
    x = nc.dram_tensor("x", xshape, bf16, kind="ExternalInput").ap()
    amats = nc.dram_tensor("amats", [128, DIM], bf16, kind="ExternalInput").ap()
    bmats = nc.dram_tensor("bmats", [128, DIM], bf16, kind="ExternalInput").ap()

    with tile.TileContext(nc) as tc:
        with ExitStack() as ctx:
            if variant == "v3":
                ident = nc.dram_tensor(
                    "ident", [128, 128], bf16, kind="ExternalInput").ap()
                out = nc.dram_tensor(
                    "out", [RPC, DIM], bf16, kind="ExternalOutput").ap()
                _emit_kernel_v3(ctx, tc, out, x, amats, bmats, ident)
            else:
                out = nc.dram_tensor(
                    "out", [NSLAB, 128, NB, RPS], bf16,
                    kind="ExternalOutput").ap()
                _emit_kernel_v4(ctx, tc, out, x, amats, bmats)

    _hoist_matmul_waits(nc)
    _spread_swdge_queues(nc)
    _CACHED[variant] = nc
    return nc


def make_in_maps(x, angles, variant=None):
    variant = variant or VARIANT
    x = np.ascontiguousarray(np.asarray(x, np.float32)).astype(BF16)
    amats64, bmats64 = _build_mats(angles)
    amats = amats64.astype(BF16)
    bmats = bmats64.astype(BF16)
    maps = []
    for c in range(NCORES):
        xc = x[c * RPC:(c + 1) * RPC]
        if variant != "v3":
            # pretranspose/pretile: [s, p, b, r] = feature b*128+p, row s*RPS+r
            xc = np.ascontiguousarray(
                xc.reshape(NSLAB, RPS, NB, 128).transpose(0, 3, 2, 1))
        m = {"x": xc, "amats": amats, "bmats": bmats}
        if variant == "v3":
            m["ident"] = np.eye(128, dtype=BF16)
        maps.append(m)
    return maps


def _unscramble(res, variant=None):
    variant = variant or VARIANT
    outs = []
    for c in range(NCORES):
        o = np.asarray(res.results[c]["out"]).astype(np.float32)
        if variant == "v3":
            outs.append(o)
        else:
            # o[s, pl*32+b, t*RPS+r] -> row s*RPS+r, feature b*128+4t+pl
            o = o.reshape(NSLAB, 4, 32, 32, RPS)
            outs.append(np.transpose(o, (0, 4, 2, 3, 1)).reshape(RPC, DIM))
    return np.concatenate(outs, axis=0)


def run_on_hw(x, angles, trace=False, trace_kwargs=None):
    from concourse.bass_utils import run_bass_kernel_spmd
    nc = _build_bass()
    in_maps = make_in_maps(x, angles)
    res = run_bass_kernel_spmd(
        nc, in_maps, core_ids=list(range(NCORES)), trace=trace,
        **(trace_kwargs or {}))
    out = _unscramble(res)
    return out, res


def kernel(x, angles):
    last_err = None
    for attempt in range(3):
        try:
            out, _ = run_on_hw(x, angles, trace=False)
            return np.ascontiguousarray(out.astype(np.float32))
        except Exception as e:  # transient NRT/device errors: retry
            last_err = e
            import time
            time.sleep(5)
    raise last_err


# revision 32
# speedup vs baseline: 1.2674x; 1.0017x over previous
"""Butterfly (Givens) rotation network on TRN2, 8 NeuronCores.

Algorithm
---------
x: (8192, 4096) f32. 12 butterfly layers; layer l rotates pairs of features
differing in bit l of the feature index. Split into two linear stages:

  Stage A = layers 0-6: features mix only within 128-wide blocks b (bits 0-6)
            -> per-block 128x128 matrix A_b  (amats[:, 128b:128b+128] =
            A_b[f_in, f_out]).
  Stage B = layers 7-11: features mix only across blocks at fixed within-block
            position p (bits 7-11) -> per-p 32x32 matrix B_p; grouping 4
            consecutive p per 128-partition tile gives block-diag 128x128
            (bmats tile t, within-tile index n = pl*32 + b for p = 4t+pl).

Variants
--------
v1: original fp32 row-major kernel (PE transposes + data-stationary matmuls).
v3: v1 structure, all-bf16 (PE transpose 1cyc/row, matmul 1cyc/row vs 4).
v4: feature-major, DMA-xbar-transposed load (zero stage-A PE transposes),
    weights-stationary bf16 matmuls at N=256, partition-regroup between the
    stages done by a plain SBUF->SBUF DMA, output left feature-major and
    unscrambled on the host.

Sharding: data-parallel over rows, 1024 rows/core; matrices replicated.
"""

import os
import numpy as np
import ml_dtypes

BF16 = ml_dtypes.bfloat16

DIM = 4096
NL = 12
NB = 32          # 128-wide feature blocks
ROWS = 8192
NCORES = 8
RPC = ROWS // NCORES     # rows per core
NT = RPC // 128          # 128-row tiles per core

RPS = 512                # v4: rows per slab
NSLAB = RPC // RPS       # v4: slabs per core


# ---------------------------------------------------------------- host math

def _butterfly_np(x, angles):
    """float64 numpy copy of the reference butterfly."""
    x = np.asarray(x, np.float64)
    angles = np.asarray(angles, np.float64)
    B, d = x.shape
    for l in range(angles.shape[0]):
        stride = 2 ** l
        nblocks = d // (2 * stride)
        xr = x.reshape(B, nblocks, 2, stride)
        c = np.cos(angles[l]).reshape(nblocks, stride)
        s = np.sin(angles[l]).reshape(nblocks, stride)
        xi = xr[:, :, 0, :].copy()
        xj = xr[:, :, 1, :].copy()
        x = np.stack([c * xi + s * xj, -s * xi + c * xj], axis=2).reshape(B, d)
    return x


def _build_mats(angles):
    """Returns (amats, bmats) each [128, 4096] f64 in SBUF-ready layout."""
    angles = np.asarray(angles, np.float64)
    amats = np.zeros((128, DIM), np.float64)
    for b in range(NB):
        # A_b[f_in, f_out]: butterfly of identity rows = F for this block
        amats[:, 128 * b:128 * b + 128] = _butterfly_np(
            np.eye(128), angles[0:7, 64 * b:64 * b + 64])
    bmats = np.zeros((128, DIM), np.float64)
    for t in range(32):
        for pl in range(4):
            p = 4 * t + pl
            BpT = _butterfly_np(np.eye(32), angles[7:12, p::128])
            bmats[32 * pl:32 * pl + 32, 128 * t + 32 * pl:128 * t + 32 * pl + 32] = BpT
    return amats, bmats


# ---------------------------------------------------------------- bass kernels

def _emit_kernel_v3(ctx, tc, out, x, amats, bmats, ident):
    """v1 structure, all-bf16: per 128-row tile, PE-transpose each feature
    block, bf16 matmul against A (data stationary), scatter-drain into f~
    order, repeat for stage B, DMA out bf16."""
    import concourse.mybir as mybir

    nc = tc.nc
    f32 = mybir.dt.float32
    bf16 = mybir.dt.bfloat16

    consts = ctx.enter_context(tc.tile_pool(name="consts", bufs=1))
    xin = ctx.enter_context(tc.tile_pool(name="xin", bufs=3))
    ystage = ctx.enter_context(tc.tile_pool(name="ystage", bufs=3))
    ostage = ctx.enter_context(tc.tile_pool(name="ostage", bufs=3))
    sbst = ctx.enter_context(tc.tile_pool(name="sbst", bufs=6))
    psA = ctx.enter_context(tc.tile_pool(name="psA", bufs=5, space="PSUM"))
    psB = ctx.enter_context(tc.tile_pool(name="psB", bufs=3, space="PSUM"))

    am = consts.tile([128, DIM], bf16, tag="amats")
    bm = consts.tile([128, DIM], bf16, tag="bmats")
    idt = consts.tile([128, 128], bf16, tag="ident")
    nc.sync.dma_start(idt[:], ident[:])

    # Greedy least-loaded assignment of PSUM->SBUF copies to DVE/ACT,
    # using measured per-copy costs (ns) for [128,512] from PSUM.
    load = {"dve": 0.0, "act": 0.0}
    cost = {("dve", "plain"): 392, ("dve", "scatter"): 700,
            ("act", "plain"): 357, ("act", "scatter"): 1127}

    def copy(dst, src, kind="plain"):
        eng = min(("dve", "act"), key=lambda e: load[e] + cost[(e, kind)])
        load[eng] += cost[(eng, kind)]
        (nc.vector.tensor_copy if eng == "dve" else nc.scalar.copy)(dst, src)

    for i in range(NT):
        xt = xin.tile([128, DIM], bf16, tag="xt")
        if i == 0:
            # first tile: fine-grained x/amats chunk interleave so the very
            # first transposes and stage-A matmuls start as early as possible
            for c in range(8):
                nc.sync.dma_start(xt[:, 512 * c:512 * (c + 1)],
                                  x[0:128, 512 * c:512 * (c + 1)])
                nc.sync.dma_start(am[:, 512 * c:512 * (c + 1)],
                                  amats[:, 512 * c:512 * (c + 1)])
        else:
            nc.sync.dma_start(xt[:], x[128 * i:128 * (i + 1), :])
        Y = ystage.tile([128, DIM], bf16, tag="Y")

        for g in range(8):           # groups of 4 feature blocks
            pt = psA.tile([128, 512], bf16, tag="ptA")
            for j in range(4):
                b = 4 * g + j
                nc.tensor.transpose(
                    pt[:, 128 * j:128 * (j + 1)],
                    xt[:, 128 * b:128 * (b + 1)], idt[:])
            xT4 = sbst.tile([128, 512], bf16, tag="xT4")
            copy(xT4[:], pt[:])
            pm = psB.tile([128, 512], f32, tag="pmA")
            for j in range(4):
                b = 4 * g + j
                nc.tensor.matmul(
                    pm[:, 128 * j:128 * (j + 1)],
                    xT4[:, 128 * j:128 * (j + 1)],
                    am[:, 128 * b:128 * (b + 1)],
                    start=True, stop=True)
            # scatter into Y: dest f~ = t*128 + pl*32 + (4g+j), src = j*128 + 4t + pl
            src = pm[:].rearrange("r (j t pl) -> r j t pl", j=4, t=32, pl=4)
            dst = Y[:].rearrange(
                "r (t pl g j) -> r g j t pl", t=32, pl=4, g=8, j=4)[:, g]
            copy(dst, src, kind="scatter")

        if i == 0:
            for c in range(8):
                nc.sync.dma_start(bm[:, 512 * c:512 * (c + 1)],
                                  bmats[:, 512 * c:512 * (c + 1)])
        O = ostage.tile([128, DIM], bf16, tag="O")
        for g in range(8):           # groups of 4 f~ tiles
            pt = psA.tile([128, 512], bf16, tag="ptA")
            for j in range(4):
                t = 4 * g + j
                nc.tensor.transpose(
                    pt[:, 128 * j:128 * (j + 1)],
                    Y[:, 128 * t:128 * (t + 1)], idt[:])
            z4 = sbst.tile([128, 512], bf16, tag="xT4")
            copy(z4[:], pt[:])
            pm = psB.tile([128, 512], f32, tag="pmA")
            for j in range(4):
                t = 4 * g + j
                nc.tensor.matmul(
                    pm[:, 128 * j:128 * (j + 1)],
                    z4[:, 128 * j:128 * (j + 1)],
                    bm[:, 128 * t:128 * (t + 1)],
                    start=True, stop=True)
            # scatter to natural order: dest f = b*128 + 4t + pl = b*128 + 16g + 4j + pl
            src = pm[:].rearrange("r (j pl b) -> r j pl b", j=4, pl=4, b=32)
            dst = O[:].rearrange(
                "r (b g j pl) -> r g j pl b", b=32, g=8, j=4, pl=4)[:, g]
            copy(dst, src, kind="scatter")

        nc.sync.dma_start(out[128 * i:128 * (i + 1), :], O[:])


def _emit_kernel_v4(ctx, tc, out, x, amats, bmats):
    """Feature-major pipeline. Emission order matches readiness: quartered
    loads for all slabs (SP), stage A for all slabs (1 matmul + 1 one-bank
    drain per feature block, psA bufs=4), then per slab all 32 regroup DMAs
    (SP/GPSIMD split, 16-deep Z rotation) followed by stage B, stores
    (quartered, SP) last. W reuses the dead XT buffer. Host unscrambles."""
    import concourse.mybir as mybir

    nc = tc.nc
    f32 = mybir.dt.float32
    bf16 = mybir.dt.bfloat16

    consts = ctx.enter_context(tc.tile_pool(name="consts", bufs=1))
    xwp = ctx.enter_context(tc.tile_pool(name="xwp", bufs=2))
    ypool = ctx.enter_context(tc.tile_pool(name="ypool", bufs=2))
    zpool = ctx.enter_context(tc.tile_pool(name="zpool", bufs=32))
    psA = ctx.enter_context(tc.tile_pool(name="psA", bufs=6, space="PSUM"))
    psB = ctx.enter_context(tc.tile_pool(name="psB", bufs=2, space="PSUM"))

    am = consts.tile([128, DIM], bf16, tag="amats")
    bm = consts.tile([128, DIM], bf16, tag="bmats")
    nc.sync.dma_start(am[:], amats[:])
    nc.sync.dma_start(bm[:], bmats[:])

    QC = NB // 4                   # feature blocks per quarter (stores)
    EC = NB // 8                   # feature blocks per eighth (loads)
    XTs = []
    for s in range(NSLAB):
        XT = xwp.tile([128, NB * RPS], bf16, tag="XW", name=f"XT{s}")
        for h in range(8):
            nc.sync.dma_start(XT[:, EC * RPS * h:EC * RPS * (h + 1)],
                              x[s, :, EC * h:EC * (h + 1)])
        XTs.append(XT)

    load = {"dve": 0.0, "act": 0.0}
    cost = {"dve": 1.2, "act": 1.0}

    def copy(dst, src):
        eng = min(("dve", "act"), key=lambda e: load[e] + cost[e])
        load[eng] += cost[eng]
        (nc.vector.tensor_copy if eng == "dve" else nc.scalar.copy)(dst, src)

    Ys = []
    for s in range(NSLAB):
        XT = XTs[s]
        Y = ypool.tile([128, NB * RPS], bf16, tag="Y", name=f"Y{s}")
        for b in range(NB):
            pm = psA.tile([128, RPS], f32, tag="pmA")
            nc.tensor.matmul(
                pm[:], am[:, 128 * b:128 * (b + 1)],
                XT[:, RPS * b:RPS * (b + 1)], start=True, stop=True)
            copy(Y[:, RPS * b:RPS * (b + 1)], pm[:])
        Ys.append(Y)

    Ws = []
    for s in range(NSLAB):
        Y = Ys[s]
        zs = []
        for t in range(NB):
            Zt = zpool.tile([128, RPS], bf16, tag="Zt")
            eng = nc.sync if t % 4 == 0 else nc.gpsimd
            eng.dma_start(Zt[:], Y[4 * t:4 * t + 4, :])
            zs.append(Zt)
        W = xwp.tile([128, NB * RPS], bf16, tag="XW", name=f"W{s}")
        for t in range(NB):
            pm = psB.tile([128, RPS], f32, tag="pmB")
            nc.tensor.matmul(
                pm[:], bm[:, 128 * t:128 * (t + 1)], zs[t][:],
                start=True, stop=True)
            copy(W[:, RPS * t:RPS * (t + 1)], pm[:])
        Ws.append(W)

    for s in range(NSLAB):
        for h in range(8):
            nc.sync.dma_start(out[s, :, EC * h:EC * (h + 1)],
                              Ws[s][:, EC * RPS * h:EC * RPS * (h + 1)])


def _hoist_matmul_waits(nc):
    """Walrus's fp32/transpose matmul (self-loading LDWEIGHTS) accepts fewer
    sync waits than Tile may assign. Hoist multi-waits onto a PE NoOp inserted
    just before the matmul — same engine queue, so ordering is identical."""
    import concourse.mybir as mybir

    n_hoisted = 0
    for blk in nc.m.functions[0].blocks:
        il = blk.instructions
        i = 0
        while i < len(il):
            inst = il[i]
            si = inst.sync_info
            if (si is not None and len(si.on_wait) > 1
                    and not isinstance(inst, mybir.InstNoOp)):
                waits = list(si.on_wait)
                # keep the last wait on the matmul; one NoOp per extra wait
                # (cayman instructions carry at most one sem-wait each)
                for k, w in enumerate(waits[:-1]):
                    nop = mybir.InstNoOp(
                        name=f"{inst.name}_hw{k}", engine=inst.engine,
                        bass_nofuse=True)
                    nop.sync_info = mybir.SyncInfo(on_wait=[w], on_update=[])
                    nc.register_instruction(nop, overwrite=True)
                    il.insert(i, nop)
                    i += 1
                    n_hoisted += 1
                inst.sync_info = mybir.SyncInfo(
                    on_wait=[waits[-1]], on_update=list(si.on_update))
            i += 1
    return n_hoisted


def _spread_swdge_queues(nc):
    """Round-robin Pool-engine DMA copies across the SWDGE rings so the SDMA
    engines interleave two regroups (packet-granular round-robin between
    queues) and their partition-concentrated reads spread across ports."""
    import concourse.mybir as mybir

    n = 0
    for blk in nc.m.functions[0].blocks:
        for inst in blk.instructions:
            if (isinstance(inst, mybir.InstDMACopy)
                    and inst.queue == "qPoolDynamic"):
                if n % 4:
                    inst.queue = f"qPoolDynamic{n % 4}"
                n += 1
    return n


_CACHED = {}
VARIANT = os.environ.get("BFLY_VARIANT", "v4")   # "v3" | "v4"


def _build_bass(variant=None):
    variant = variant or VARIANT
    if variant in _CACHED:
        return _CACHED[variant]
    from contextlib import ExitStack
    import concourse.bass as bass
    import concourse.tile as tile
    import concourse.mybir as mybir

    bf16 = mybir.dt.bfloat16
    nc = bass.Bass("TRN2", target_bir_lowering=False, debug=False,
                   num_devices=NCORES, num_swdge_queues=4)
    xshape = [RPC, DIM] if variant == "v3" else [NSLAB, 128, NB, RPS]
    x = nc.dram_tensor("x", xshape, bf16, kind="ExternalInput").ap()
    amats = nc.dram_tensor("amats", [128, DIM], bf16, kind="ExternalInput").ap()
    bmats = nc.dram_tensor("bmats", [128, DIM], bf16, kind="ExternalInput").ap()

    with tile.TileContext(nc) as tc:
        with ExitStack() as ctx:
            if variant == "v3":
                ident = nc.dram_tensor(
                    "ident", [128, 128], bf16, kind="ExternalInput").ap()
                out = nc.dram_tensor(
                    "out", [RPC, DIM], bf16, kind="ExternalOutput").ap()
                _emit_kernel_v3(ctx, tc, out, x, amats, bmats, ident)
            else:
                out = nc.dram_tensor(
                    "out", [NSLAB, 128, NB, RPS], bf16,
                    kind="ExternalOutput").ap()
                _emit_kernel_v4(ctx, tc, out, x, amats, bmats)

    _hoist_matmul_waits(nc)
    _spread_swdge_queues(nc)
    _CACHED[variant] = nc
    return nc


def make_in_maps(x, angles, variant=None):
    variant = variant or VARIANT
    x = np.ascontiguousarray(np.asarray(x, np.float32)).astype(BF16)
    amats64, bmats64 = _build_mats(angles)
    amats = amats64.astype(BF16)
    bmats = bmats64.astype(BF16)
    maps = []
    for c in range(NCORES):
        xc = x[c * RPC:(c + 1) * RPC]
        if variant != "v3":
            # pretranspose/pretile: [s, p, b, r] = feature b*128+p, row s*RPS+r
            xc = np.ascontiguousarray(
                xc.reshape(NSLAB, RPS, NB, 128).transpose(0, 3, 2, 1))
        m = {"x": xc, "amats": amats, "bmats": bmats}
        if variant == "v3":
            m["ident"] = np.eye(128, dtype=BF16)
        maps.append(m)
    return maps


def _unscramble(res, variant=None):
    variant = variant or VARIANT
    outs = []
    for c in range(NCORES):
        o = np.asarray(res.results[c]["out"]).astype(np.float32)
        if variant == "v3":
            outs.append(o)
        else:
            # o[s, pl*32+b, t*RPS+r] -> row s*RPS+r, feature b*128+4t+pl
            o = o.reshape(NSLAB, 4, 32, 32, RPS)
            outs.append(np.transpose(o, (0, 4, 2, 3, 1)).reshape(RPC, DIM))
    return np.concatenate(outs, axis=0)


def run_on_hw(x, angles, trace=False, trace_kwargs=None):
    from concourse.bass_utils import run_bass_kernel_spmd
    nc = _build_bass()
    in_maps = make_in_maps(x, angles)
    res = run_bass_kernel_spmd(
        nc, in_maps, core_ids=list(range(NCORES)), trace=trace,
        **(trace_kwargs or {}))
    out = _unscramble(res)
    return out, res


def kernel(x, angles):
    last_err = None
    for attempt in range(3):
        try:
            out, _ = run_on_hw(x, angles, trace=False)
            return np.ascontiguousarray(out.astype(np.float32))
        except Exception as e:  # transient NRT/device errors: retry
            last_err = e
            import time
            time.sleep(5)
    raise last_err
